# revision 18
# baseline (speedup 1.0000x reference)
"""Trainium2 Bass kernel for nn_CausalMemory (reverse-causal decayed attention).

Math: out = ((qh @ xb.T) * W) @ xb @ VOB, where xb = x @ basis (rank-128),
qh = xb @ (Qc.T Kc), VOB = (Vc.T Oc) basis.T * out_scale, and
W[t,s] = decay^(s-t-1) for s>t else 0 (strictly-future attention).
decay^256 ~ 4e-6, so attention is windowed to the next J-1 chunks of 128.

Sharding: 8 cores = batch(4) x sequence-halves(2). Each core handles 2048
query tokens; its key/value range extends (J-1)*128 tokens past the query
range (zero-padded at the end of the sequence, which reproduces truncation
exactly).

Output is produced in [C, T] (transposed) layout so the final projection can
keep VOB slices stationary in the PE array; the host transposes back.

rv accumulation uses PSUM has_written semantics: per 512-column PSUM bank,
the first matmul (start=True) clears the bank, subsequent matmuls with
start=False accumulate where previous matmuls wrote and overwrite elsewhere,
letting one N=256 matmul per key chunk serve two query chunks.
"""

import numpy as np
import ml_dtypes

B, T, C, H = 4, 4096, 512, 128
TQ = 2048           # query tokens per core
CH = 128            # chunk
J = 2               # window chunks (own + J-1 ahead)
LOOK = (J - 1) * CH
TK = TQ + LOOK      # key tokens per core
NCH = TK // CH      # key chunks per core
NT = TQ // CH       # query tiles per core

# input token blocks; first blocks small so the first matmuls start ASAP
KBW = [128, 384, 512, 512, 512, 128]
assert sum(KBW) == TK
NKB = len(KBW)
KBO = [sum(KBW[:b]) for b in range(NKB)]  # block token offsets

# output token blocks (small tail blocks shorten the critical path)
OBW = [512, 512, 512, 256, 256]
assert sum(OBW) == TQ
OBO = [sum(OBW[:b]) for b in range(len(OBW))]

_CACHE = {}

CFG = {
    "warm": 6,         # warmup matmuls (N=512) to spin HAM to 8/8
    "xtok_eng": "v",
    "qh_eng": "s",
    "xb_eng": "s",
    "dma_s": "sync",   # output stores
    "late_ld": "gp",   # engine for non-critical late input loads
    "n_sync_ld": 3,    # xt blocks 0..n-1 load via sync, rest via late_ld
}


def _build():
    import concourse.tile as tile
    from concourse import bacc, mybir

    bf16 = mybir.dt.bfloat16
    f32 = mybir.dt.float32

    nc = bacc.Bacc("TRN2", target_bir_lowering=False, debug=False, num_devices=8)

    xt_ext = nc.declare_dram_parameter("xt", [128, 4 * TK], bf16, isOutput=False)
    c0_ext = nc.declare_dram_parameter("c0", [128, 512], bf16, isOutput=False)
    ca_ext = nc.declare_dram_parameter("ca", [128, 256], bf16, isOutput=False)
    c1_ext = nc.declare_dram_parameter("c1", [128, 512 + J * 128], bf16,
                                       isOutput=False)
    out_ext = nc.declare_dram_parameter("out", [512, TQ], bf16, isOutput=True)

    def _dma_l(dst, srcap, late=False):
        eng = nc.gpsimd if (late and CFG["late_ld"] == "gp") else nc.sync
        eng.dma_start(dst, srcap)

    def _dma_s(dst, srcap):
        eng = nc.sync if CFG["dma_s"] == "sync" else nc.gpsimd
        eng.dma_start(dst, srcap)

    def _copy(eng, dst, srcap):
        if eng == "v":
            nc.vector.tensor_copy(dst, srcap)
        elif eng == "s":
            nc.scalar.copy(dst, srcap)
        else:
            nc.any.tensor_copy(dst, srcap)

    with tile.TileContext(nc) as tc:
        with (
            tc.tile_pool(name="consts", bufs=1) as cpool,
            tc.tile_pool(name="xt", bufs=3) as xtp,
            tc.tile_pool(name="big", bufs=1) as bigp,
            tc.tile_pool(name="ps_xb", bufs=2, space="PSUM") as ps_xb,
            tc.tile_pool(name="ps_tq", bufs=2, space="PSUM") as ps_tq,
            tc.tile_pool(name="ps_st", bufs=1, space="PSUM") as ps_stp,
            tc.tile_pool(name="ps_rv", bufs=1, space="PSUM") as ps_rvp,
            tc.tile_pool(name="ps_out", bufs=2, space="PSUM") as ps_outp,
        ):
            # ---- constants (basis first: needed by the very first matmul)
            c0 = cpool.tile([128, 512], bf16)
            _dma_l(c0[:], c0_ext[:])
            ca = cpool.tile([128, 256], bf16)
            c1 = cpool.tile([128, 512 + J * 128], bf16)

            # ---- warmup: spin the PE HAM clock gate up while input DMAs fly
            if CFG["warm"]:
                warm = cpool.tile([128, 640], bf16)
                nc.gpsimd.memset(warm[:], 0)
                for wi in range(CFG["warm"]):
                    pwarm = ps_outp.tile([128, 512], f32, tag="pout")
                    nc.tensor.matmul(pwarm[:], warm[:, 0:128], warm[:, 128:640],
                                     start=True, stop=True)

            basis_s = c0[:, 0:512]
            a_s = ca[:, 0:128]
            id_s = ca[:, 128:256]
            vob_s = c1[:, 0:512]
            wm_s = c1[:, 512:512 + J * 128]

            xb_big = bigp.tile([128, TK], bf16, tag="xb")
            xtok_big = bigp.tile([128, TK], bf16, tag="xtok")
            qh_big = bigp.tile([128, TQ], bf16, tag="qh")
            rv_big = bigp.tile([128, TQ], bf16, tag="rv")
            st_big = bigp.tile([128, NCH, J * 128], bf16, tag="stb")
            out_sb = bigp.tile([128, 4, TQ], bf16, tag="outsb")

            def block_stage(kb):
                w = KBW[kb]
                off = KBO[kb]
                xt3 = xtp.tile([128, 4, w], bf16, tag="xt")
                _dma_l(
                    xt3[:],
                    xt_ext[:, 4 * off:4 * off + 4 * w].rearrange(
                        "p (s t) -> p s t", s=4),
                    late=(kb >= CFG["n_sync_ld"]))
                if kb == 0:
                    # A / identity: needed right after the first xb tile
                    _dma_l(ca[:], ca_ext[:])
                if kb == 1:
                    # vob / wmask: needed ~mid-kernel
                    _dma_l(c1[:], c1_ext[:])
                for g0 in range(0, w, 512):
                    gw = min(512, w - g0)
                    pxb = ps_xb.tile([128, gw], f32, tag="pxb")
                    for sl in range(4):
                        nc.tensor.matmul(
                            pxb[:], basis_s[:, sl * 128:(sl + 1) * 128],
                            xt3[:, sl, g0:g0 + gw],
                            start=(sl == 0), stop=(sl == 3))
                    xb = xb_big[:, off + g0:off + g0 + gw]
                    _copy(CFG["xb_eng"], xb, pxb[:])

                    ptk = ps_tq.tile([128, gw], bf16, tag="ptq", name="ptk")
                    for ci in range(gw // 128):
                        nc.tensor.transpose(
                            ptk[:, ci * 128:(ci + 1) * 128],
                            xb[:, ci * 128:(ci + 1) * 128], id_s)
                    _copy(CFG["xtok_eng"],
                          xtok_big[:, off + g0:off + g0 + gw], ptk[:])

                    qoff = off + g0
                    if qoff < TQ:
                        qw = min(gw, TQ - qoff)
                        pqh = ps_tq.tile([128, qw], f32, tag="ptq", name="pqh")
                        nc.tensor.matmul(pqh[:], a_s, xb[:, :qw],
                                         start=True, stop=True)
                        _copy(CFG["qh_eng"], qh_big[:, qoff:qoff + qw], pqh[:])

            pst_pair = {}

            def scores_stage(c):
                n0 = max(0, c - (J - 1))
                n1 = min(NT - 1, c)
                L = n1 - n0 + 1
                # two chunks share one PSUM bank (tiles are half-bank sized)
                if c % 2 == 0:
                    pst_pair[c // 2] = ps_stp.tile([128, 2, J * 128], f32,
                                                   tag="pst", name="pstp")
                pst = pst_pair[c // 2][:, c % 2, :]
                nc.tensor.matmul(
                    pst[:, :L * 128],
                    xb_big[:, c * 128:(c + 1) * 128],
                    qh_big[:, n0 * 128:(n1 + 1) * 128],
                    start=True, stop=True)
                # wm slab k holds j=J-1-k; the needed j run (c-n0 .. c-n1) is a
                # contiguous slice of it
                w0 = (J - 1 - (c - n0)) * 128
                nc.vector.tensor_mul(st_big[:, c, :L * 128], pst[:, :L * 128],
                                     wm_s[:, w0:w0 + L * 128])

            def rv_bank(g):
                # query chunks 4g..4g+3 accumulate in one PSUM bank via
                # has_written: contributions from key chunks 4g..4g+4.
                prv = ps_rvp.tile([128, 512], f32, tag="prv")
                first = True
                for c in range(4 * g, 4 * g + 5):
                    if c > NCH - 1:
                        break
                    n0 = max(0, c - (J - 1))
                    # query chunks covered by st[c]: n0..min(NT-1, c); clip to
                    # this bank's range 4g..4g+3
                    q_lo = max(n0, 4 * g)
                    q_hi = min(min(NT - 1, c), 4 * g + 3)
                    if q_lo > q_hi:
                        continue
                    p0 = q_lo - n0
                    nq = q_hi - q_lo + 1
                    nc.tensor.matmul(
                        prv[:, (q_lo - 4 * g) * 128:(q_hi - 4 * g + 1) * 128],
                        xtok_big[:, c * 128:(c + 1) * 128],
                        st_big[:, c, p0 * 128:(p0 + nq) * 128],
                        start=first, stop=(c == min(4 * g + 4, NCH - 1)),
                        skip_group_check=True)
                    first = False
                _copy("v" if g % 2 == 0 else "s",
                      rv_big[:, g * 512:(g + 1) * 512], prv[:])

            def out_block(b):
                toff, w = OBO[b], OBW[b]
                for cs in range(4):
                    pout = ps_outp.tile([128, 512], f32, tag="pout")
                    nc.tensor.matmul(pout[:, :w],
                                     vob_s[:, cs * 128:(cs + 1) * 128],
                                     rv_big[:, toff:toff + w],
                                     start=True, stop=True)
                    _copy("v" if cs % 2 == b % 2 else "s",
                          out_sb[:, cs, toff:toff + w], pout[:, :w])
                _dma_s(
                    out_ext[:, toff:toff + w].rearrange(
                        "(s p) t -> p s t", p=128),
                    out_sb[:, :, toff:toff + w])

            emitted_c = 0
            emitted_g = 0
            emitted_b = 0
            for kb in range(NKB):
                block_stage(kb)
                chunks_done = (KBO[kb] + KBW[kb]) // 128
                while emitted_c < min(chunks_done, NCH):
                    scores_stage(emitted_c)
                    emitted_c += 1
                while emitted_g < NT // 4 and \
                        min(4 * emitted_g + 4, NCH - 1) < emitted_c:
                    rv_bank(emitted_g)
                    emitted_g += 1
                while emitted_b < len(OBW) and \
                        OBO[emitted_b] + OBW[emitted_b] <= emitted_g * 512:
                    out_block(emitted_b)
                    emitted_b += 1
            while emitted_c < NCH:
                scores_stage(emitted_c)
                emitted_c += 1
            while emitted_g < NT // 4:
                rv_bank(emitted_g)
                emitted_g += 1
            while emitted_b < len(OBW):
                out_block(emitted_b)
                emitted_b += 1

    nc.compile()
    return nc


def _host_consts(basis, qc, kc, vc, oc, decay_logit, out_scale):
    bf = ml_dtypes.bfloat16
    d = 1.0 / (1.0 + np.exp(-np.float64(decay_logit)))
    basis64 = np.asarray(basis, np.float64)
    A = np.asarray(qc, np.float64).T @ np.asarray(kc, np.float64)
    VOB = (np.asarray(vc, np.float64).T @ np.asarray(oc, np.float64)) \
        @ basis64.T * np.float64(out_scale)
    # full decay mask, reversed slab order: slab k holds j = J-1-k.
    # value at [s, k*128+t] = d^(128j + s - t - 1) for j>=1;  j=0: tri.
    W = np.zeros((CH, J * CH), dtype=np.float64)
    s_idx = np.arange(CH)[:, None]
    t_idx = np.arange(CH)[None, :]
    for k in range(J):
        j = J - 1 - k
        if j == 0:
            W[:, k * CH:(k + 1) * CH] = np.where(
                s_idx > t_idx, d ** np.maximum(s_idx - t_idx - 1, 0), 0.0)
        else:
            W[:, k * CH:(k + 1) * CH] = d ** (CH * j + s_idx - t_idx - 1)

    c0 = basis64.astype(np.float32).reshape(4, 128, 128) \
        .transpose(1, 0, 2).reshape(128, 512).astype(bf)
    ca = np.zeros((128, 256), dtype=bf)
    ca[:, 0:128] = A.astype(np.float32).astype(bf)
    ca[:, 128:256] = np.eye(128, dtype=np.float32).astype(bf)
    c1 = np.zeros((128, 512 + J * CH), dtype=bf)
    c1[:, 0:512] = VOB.astype(np.float32).astype(bf)
    c1[:, 512:512 + J * CH] = W.astype(np.float32).astype(bf)
    return c0, ca, c1


def make_in_maps(x, basis, q_coeffs, k_coeffs, v_coeffs, o_coeffs,
                 decay_logit, out_scale):
    bf = ml_dtypes.bfloat16
    c0, ca, c1 = _host_consts(basis, q_coeffs, k_coeffs, v_coeffs, o_coeffs,
                              decay_logit, out_scale)
    x = np.asarray(x, np.float32)
    in_maps = []
    for b in range(B):
        xbT = np.ascontiguousarray(x[b].T)  # [C, T]
        for h in range(2):
            q0 = h * TQ
            xs = np.zeros((C, TK), dtype=np.float32)
            avail = min(TK, T - q0)
            xs[:, :avail] = xbT[:, q0:q0 + avail]
            x4 = xs.reshape(4, 128, TK)
            xt_p = np.empty((128, 4 * TK), dtype=bf)
            for kb in range(NKB):
                off, w = KBO[kb], KBW[kb]
                xt_p[:, 4 * off:4 * off + 4 * w] = (
                    x4[:, :, off:off + w].transpose(1, 0, 2).reshape(128, 4 * w))
            in_maps.append({"xt": xt_p, "c0": c0, "ca": ca, "c1": c1})
    return in_maps


def assemble_out(results):
    out = np.zeros((B, T, C), dtype=np.float32)
    for core in range(8):
        b, h = core // 2, core % 2
        out[b, h * TQ:(h + 1) * TQ, :] = np.asarray(
            results[core]["out"]).astype(np.float32).T
    return out


def get_nc():
    if "nc" not in _CACHE:
        _CACHE["nc"] = _build()
    return _CACHE["nc"]


def kernel(x, basis, q_coeffs, k_coeffs, v_coeffs, o_coeffs,
           decay_logit, out_scale):
    from concourse.bass_utils import run_bass_kernel_spmd

    nc = get_nc()
    in_maps = make_in_maps(x, basis, q_coeffs, k_coeffs, v_coeffs, o_coeffs,
                           decay_logit, out_scale)
    res = run_bass_kernel_spmd(nc, in_maps, list(range(8)))
    return assemble_out(res.results)


# revision 29
# speedup vs baseline: 1.0622x; 1.0622x over previous
"""Trainium2 Bass kernel for nn_CausalMemory (reverse-causal decayed attention).

Math: out = ((qh @ xb.T) * W) @ xb @ VOB, where xb = x @ basis (rank-128),
qh = xb @ (Qc.T Kc), VOB = (Vc.T Oc) basis.T * out_scale, and
W[t,s] = decay^(s-t-1) for s>t else 0 (strictly-future attention).
decay^256 ~ 4e-6, so attention is windowed to the next J-1 chunks of 128.

Sharding: 8 cores = batch(4) x sequence-halves(2). Each core handles 2048
query tokens; its key/value range extends (J-1)*128 tokens past the query
range (zero-padded at the end of the sequence, which reproduces truncation
exactly).

Output is produced in [C, T] (transposed) layout so the final projection can
keep VOB slices stationary in the PE array; the host transposes back.

rv accumulation uses PSUM has_written semantics: per 512-column PSUM bank,
the first matmul (start=True) clears the bank, subsequent matmuls with
start=False accumulate where previous matmuls wrote and overwrite elsewhere,
letting one N=256 matmul per key chunk serve two query chunks.
"""

import numpy as np
import ml_dtypes

B, T, C, H = 4, 4096, 512, 128
TQ = 2048           # query tokens per core
CH = 128            # chunk
J = 2               # window chunks (own + J-1 ahead)
LOOK = (J - 1) * CH
TK = TQ + LOOK      # key tokens per core
NCH = TK // CH      # key chunks per core
NT = TQ // CH       # query tiles per core

# input token blocks; first blocks small so the first matmuls start ASAP
KBW = [128, 384, 512, 512, 512, 128]
assert sum(KBW) == TK
NKB = len(KBW)
KBO = [sum(KBW[:b]) for b in range(NKB)]  # block token offsets

# output token blocks (small tail blocks shorten the critical path)
OBW = [512, 512, 512, 256, 256]
assert sum(OBW) == TQ
OBO = [sum(OBW[:b]) for b in range(len(OBW))]

_CACHE = {}

CFG = {
    "warm": 5,         # warmup matmuls (N=512) to spin HAM to 8/8
    "xtok_eng": "v",
    "qh_eng": "s",
    "xb_eng": "s",
    "dma_s": "sync",   # output stores
    "late_ld": "gp",   # engine for non-critical late input loads
    "n_sync_ld": 99,   # xt blocks 0..n-1 load via sync, rest via late_ld
}


def _build():
    import concourse.tile as tile
    from concourse import bacc, mybir

    bf16 = mybir.dt.bfloat16
    f32 = mybir.dt.float32

    nc = bacc.Bacc("TRN2", target_bir_lowering=False, debug=False, num_devices=8)

    # xt holds [basis | A | ident] (768 cols) followed by the x blocks, so one
    # leading DMA delivers everything the first matmuls need.
    xt_ext = nc.declare_dram_parameter("xt", [128, 768 + 4 * TK], bf16,
                                       isOutput=False)
    c1_ext = nc.declare_dram_parameter("c1", [128, 512 + 2 * J * 128], bf16,
                                       isOutput=False)
    out_ext = nc.declare_dram_parameter("out", [512, TQ], bf16, isOutput=True)

    def _dma_l(dst, srcap, late=False):
        eng = nc.gpsimd if (late and CFG["late_ld"] == "gp") else nc.sync
        eng.dma_start(dst, srcap)

    def _dma_s(dst, srcap):
        eng = nc.sync if CFG["dma_s"] == "sync" else nc.gpsimd
        eng.dma_start(dst, srcap)

    def _copy(eng, dst, srcap):
        if eng == "v":
            nc.vector.tensor_copy(dst, srcap)
        elif eng == "s":
            nc.scalar.copy(dst, srcap)
        else:
            nc.any.tensor_copy(dst, srcap)

    with tile.TileContext(nc) as tc:
        with (
            tc.tile_pool(name="consts", bufs=1) as cpool,
            tc.tile_pool(name="xt", bufs=3) as xtp,
            tc.tile_pool(name="big", bufs=1) as bigp,
            tc.tile_pool(name="ps_xb", bufs=2, space="PSUM") as ps_xb,
            tc.tile_pool(name="ps_tq", bufs=2, space="PSUM") as ps_tq,
            tc.tile_pool(name="ps_st", bufs=1, space="PSUM") as ps_stp,
            tc.tile_pool(name="ps_rv", bufs=1, space="PSUM") as ps_rvp,
            tc.tile_pool(name="ps_out", bufs=2, space="PSUM") as ps_outp,
        ):
            # ---- leading DMA: [basis | A | ident | x block 0] in one shot
            c0big = cpool.tile([128, 768 + 4 * KBW[0]], bf16)
            _dma_l(c0big[:], xt_ext[:, 0:768 + 4 * KBW[0]])
            c1 = cpool.tile([128, 512 + 2 * J * 128], bf16)

            # ---- warmup: spin the PE HAM clock gate up while input DMAs fly
            if CFG["warm"]:
                warm = cpool.tile([128, 640], bf16)
                nc.gpsimd.memset(warm[:], 0)
                for wi in range(CFG["warm"]):
                    pwarm = ps_outp.tile([128, 512], f32, tag="pout")
                    nc.tensor.matmul(pwarm[:], warm[:, 0:128], warm[:, 128:640],
                                     start=True, stop=True)

            basis_s = c0big[:, 0:512]
            a_s = c0big[:, 512:640]
            id_s = c0big[:, 640:768]
            xt0_view = c0big[:, 768:768 + 4 * KBW[0]].rearrange(
                "p (s t) -> p s t", s=4)
            vob_s = c1[:, 0:512]
            wm_s = c1[:, 512:512 + J * 128]
            wm2_s = c1[:, 512:512 + 2 * J * 128]

            xb_big = bigp.tile([128, TK], bf16, tag="xb")
            xtok_big = bigp.tile([128, TK], bf16, tag="xtok")
            qh_big = bigp.tile([128, TQ], bf16, tag="qh")
            rv_big = bigp.tile([128, TQ], bf16, tag="rv")
            st_big = bigp.tile([128, NCH, J * 128], bf16, tag="stb")
            out_sb = bigp.tile([128, 4, TQ], bf16, tag="outsb")

            def block_stage(kb):
                w = KBW[kb]
                off = KBO[kb]
                if kb == 0:
                    xt3 = xt0_view
                else:
                    xt3 = xtp.tile([128, 4, w], bf16, tag="xt")
                    _dma_l(
                        xt3[:],
                        xt_ext[:, 768 + 4 * off:768 + 4 * off + 4 * w].rearrange(
                            "p (s t) -> p s t", s=4),
                        late=(kb >= CFG["n_sync_ld"]))
                if kb == 2:
                    # vob / wmask: needed ~mid-kernel
                    _dma_l(c1[:], c1_ext[:])
                for g0 in range(0, w, 512):
                    gw = min(512, w - g0)
                    pxb = ps_xb.tile([128, gw], f32, tag="pxb")
                    for sl in range(4):
                        nc.tensor.matmul(
                            pxb[:], basis_s[:, sl * 128:(sl + 1) * 128],
                            xt3[:, sl, g0:g0 + gw],
                            start=(sl == 0), stop=(sl == 3))
                    xb = xb_big[:, off + g0:off + g0 + gw]
                    _copy(CFG["xb_eng"], xb, pxb[:])

                    ptk = ps_tq.tile([128, gw], bf16, tag="ptq", name="ptk")
                    for ci in range(gw // 128):
                        nc.tensor.transpose(
                            ptk[:, ci * 128:(ci + 1) * 128],
                            xb[:, ci * 128:(ci + 1) * 128], id_s)
                    _copy(CFG["xtok_eng"],
                          xtok_big[:, off + g0:off + g0 + gw], ptk[:])

                    qoff = off + g0
                    if qoff < TQ:
                        qw = min(gw, TQ - qoff)
                        pqh = ps_tq.tile([128, qw], f32, tag="ptq", name="pqh")
                        nc.tensor.matmul(pqh[:], a_s, xb[:, :qw],
                                         start=True, stop=True)
                        _copy(CFG["qh_eng"], qh_big[:, qoff:qoff + qw], pqh[:])

            pst_pair = {}

            def scores_stage(c):
                n0 = max(0, c - (J - 1))
                n1 = min(NT - 1, c)
                L = n1 - n0 + 1
                # two chunks share one PSUM bank (tiles are half-bank sized)
                if c % 2 == 0:
                    pst_pair[c // 2] = ps_stp.tile([128, 2, J * 128], f32,
                                                   tag="pst", name="pstp")
                pair = pst_pair[c // 2]
                pst = pair[:, c % 2, :]
                nc.tensor.matmul(
                    pst[:, :L * 128],
                    xb_big[:, c * 128:(c + 1) * 128],
                    qh_big[:, n0 * 128:(n1 + 1) * 128],
                    start=True, stop=True)
                # wm slab k holds j=J-1-k; the needed j run (c-n0 .. c-n1) is a
                # contiguous slice of it
                w0 = (J - 1 - (c - n0)) * 128
                if 2 <= c < 16 and c % 2 == 1:
                    # fuse both chunks of this bank pair into one DVE op
                    nc.vector.tensor_mul(
                        st_big[:, c - 1:c + 1, :],
                        pair[:],
                        wm2_s[:].rearrange("p (two f) -> p two f", two=2))
                elif c < 2 or c == 16:
                    nc.vector.tensor_mul(st_big[:, c, :L * 128],
                                         pst[:, :L * 128],
                                         wm_s[:, w0:w0 + L * 128])

            def rv_bank(g):
                # query chunks 4g..4g+3 accumulate in one PSUM bank via
                # has_written: contributions from key chunks 4g..4g+4.
                prv = ps_rvp.tile([128, 512], f32, tag="prv")
                first = True
                for c in range(4 * g, 4 * g + 5):
                    if c > NCH - 1:
                        break
                    n0 = max(0, c - (J - 1))
                    # query chunks covered by st[c]: n0..min(NT-1, c); clip to
                    # this bank's range 4g..4g+3
                    q_lo = max(n0, 4 * g)
                    q_hi = min(min(NT - 1, c), 4 * g + 3)
                    if q_lo > q_hi:
                        continue
                    p0 = q_lo - n0
                    nq = q_hi - q_lo + 1
                    nc.tensor.matmul(
                        prv[:, (q_lo - 4 * g) * 128:(q_hi - 4 * g + 1) * 128],
                        xtok_big[:, c * 128:(c + 1) * 128],
                        st_big[:, c, p0 * 128:(p0 + nq) * 128],
                        start=first, stop=(c == min(4 * g + 4, NCH - 1)),
                        skip_group_check=True)
                    first = False
                _copy("v" if g % 2 == 0 else "s",
                      rv_big[:, g * 512:(g + 1) * 512], prv[:])

            def out_block(b):
                toff, w = OBO[b], OBW[b]
                for cs in range(4):
                    pout = ps_outp.tile([128, 512], f32, tag="pout")
                    nc.tensor.matmul(pout[:, :w],
                                     vob_s[:, cs * 128:(cs + 1) * 128],
                                     rv_big[:, toff:toff + w],
                                     start=True, stop=True)
                    _copy("v" if cs % 2 == b % 2 else "s",
                          out_sb[:, cs, toff:toff + w], pout[:, :w])
                _dma_s(
                    out_ext[:, toff:toff + w].rearrange(
                        "(s p) t -> p s t", p=128),
                    out_sb[:, :, toff:toff + w])

            emitted_c = 0
            emitted_g = 0
            emitted_b = 0
            for kb in range(NKB):
                block_stage(kb)
                chunks_done = (KBO[kb] + KBW[kb]) // 128
                while emitted_c < min(chunks_done, NCH):
                    scores_stage(emitted_c)
                    emitted_c += 1
                while emitted_g < NT // 4 and \
                        min(4 * emitted_g + 4, NCH - 1) < emitted_c:
                    rv_bank(emitted_g)
                    emitted_g += 1
                while emitted_b < len(OBW) and \
                        OBO[emitted_b] + OBW[emitted_b] <= emitted_g * 512:
                    out_block(emitted_b)
                    emitted_b += 1
            while emitted_c < NCH:
                scores_stage(emitted_c)
                emitted_c += 1
            while emitted_g < NT // 4:
                rv_bank(emitted_g)
                emitted_g += 1
            while emitted_b < len(OBW):
                out_block(emitted_b)
                emitted_b += 1

    nc.compile()
    return nc


def _host_consts(basis, qc, kc, vc, oc, decay_logit, out_scale):
    bf = ml_dtypes.bfloat16
    d = 1.0 / (1.0 + np.exp(-np.float64(decay_logit)))
    basis64 = np.asarray(basis, np.float64)
    A = np.asarray(qc, np.float64).T @ np.asarray(kc, np.float64)
    VOB = (np.asarray(vc, np.float64).T @ np.asarray(oc, np.float64)) \
        @ basis64.T * np.float64(out_scale)
    # full decay mask, reversed slab order: slab k holds j = J-1-k.
    # value at [s, k*128+t] = d^(128j + s - t - 1) for j>=1;  j=0: tri.
    W = np.zeros((CH, J * CH), dtype=np.float64)
    s_idx = np.arange(CH)[:, None]
    t_idx = np.arange(CH)[None, :]
    for k in range(J):
        j = J - 1 - k
        if j == 0:
            W[:, k * CH:(k + 1) * CH] = np.where(
                s_idx > t_idx, d ** np.maximum(s_idx - t_idx - 1, 0), 0.0)
        else:
            W[:, k * CH:(k + 1) * CH] = d ** (CH * j + s_idx - t_idx - 1)

    hdr = np.zeros((128, 768), dtype=bf)
    hdr[:, 0:512] = basis64.astype(np.float32).reshape(4, 128, 128) \
        .transpose(1, 0, 2).reshape(128, 512).astype(bf)
    hdr[:, 512:640] = A.astype(np.float32).astype(bf)
    hdr[:, 640:768] = np.eye(128, dtype=np.float32).astype(bf)
    c1 = np.zeros((128, 512 + 2 * J * CH), dtype=bf)
    c1[:, 0:512] = VOB.astype(np.float32).astype(bf)
    Wbf = W.astype(np.float32).astype(bf)
    c1[:, 512:512 + J * CH] = Wbf
    c1[:, 512 + J * CH:512 + 2 * J * CH] = Wbf
    return hdr, c1


def make_in_maps(x, basis, q_coeffs, k_coeffs, v_coeffs, o_coeffs,
                 decay_logit, out_scale):
    bf = ml_dtypes.bfloat16
    hdr, c1 = _host_consts(basis, q_coeffs, k_coeffs, v_coeffs, o_coeffs,
                           decay_logit, out_scale)
    x = np.asarray(x, np.float32)
    in_maps = []
    for b in range(B):
        xbT = np.ascontiguousarray(x[b].T)  # [C, T]
        for h in range(2):
            q0 = h * TQ
            xs = np.zeros((C, TK), dtype=np.float32)
            avail = min(TK, T - q0)
            xs[:, :avail] = xbT[:, q0:q0 + avail]
            x4 = xs.reshape(4, 128, TK)
            xt_p = np.empty((128, 768 + 4 * TK), dtype=bf)
            xt_p[:, 0:768] = hdr
            for kb in range(NKB):
                off, w = KBO[kb], KBW[kb]
                xt_p[:, 768 + 4 * off:768 + 4 * off + 4 * w] = (
                    x4[:, :, off:off + w].transpose(1, 0, 2).reshape(128, 4 * w))
            in_maps.append({"xt": xt_p, "c1": c1})
    return in_maps


def assemble_out(results):
    out = np.zeros((B, T, C), dtype=np.float32)
    for core in range(8):
        b, h = core // 2, core % 2
        out[b, h * TQ:(h + 1) * TQ, :] = np.asarray(
            results[core]["out"]).astype(np.float32).T
    return out


def get_nc():
    if "nc" not in _CACHE:
        _CACHE["nc"] = _build()
    return _CACHE["nc"]


def kernel(x, basis, q_coeffs, k_coeffs, v_coeffs, o_coeffs,
           decay_logit, out_scale):
    from concourse.bass_utils import run_bass_kernel_spmd

    nc = get_nc()
    in_maps = make_in_maps(x, basis, q_coeffs, k_coeffs, v_coeffs, o_coeffs,
                           decay_logit, out_scale)
    res = run_bass_kernel_spmd(nc, in_maps, list(range(8)))
    return assemble_out(res.results)
